# revision 1
# baseline (speedup 1.0000x reference)
"""Causal self-attention (B=1, T=4096, D=1024, H=16, HD=64) on 8 trn2 NeuronCores.

Sharding: tensor-parallel over heads (2 heads per core) for QKV + attention;
on-chip AllToAll re-shards to sequence-parallel for the output projection
(each core computes a 512-row slice of the output).

Matmul layout notes (PE computes out = lhsT.T @ rhs, contraction on partitions):
 - host feeds x transposed (xT [D, T]) so QKV needs no on-chip transposes.
 - S^T tiles [tk, tq] are computed (not S) so the PV matmul can consume
   exp(S^T) directly as the moving operand with V in natural [tk, hd] layout.
   The two heads' QK matmuls run concurrently via PE row tiling (K=64 each).
 - a ones-column appended to V makes row 64 of the PV accumulator the
   softmax denominator (no extra reduction pass).
 - softmax max-subtraction is skipped: scores are ~N(0,1) (|s| < ~10), and
   a constant shift cancels exactly in softmax, so exp is safe in fp32.
"""

import math
import sys
from contextlib import ExitStack

sys.path.insert(0, "/opt/trn_rl_repo")

import ml_dtypes
import numpy as np

import concourse.bass as bass  # noqa: F401  (bass types used via tile/bacc)
import concourse.mybir as mybir
import concourse.tile as tile
from concourse import bacc
from concourse.bass_utils import run_bass_kernel_spmd

B, T, D, H, HD = 1, 4096, 1024, 16, 64
NCORES = 8
HPC = H // NCORES          # heads per core = 2
E = HPC * HD               # per-core head width = 128
TQ = 512                   # tq block width
NB = T // TQ               # 8 tq blocks
CK = 128                   # tk chunk (partition dim of S^T tiles)
KD = D // 128              # 8 contraction chunks over D
NV = T // CK               # 32 tk chunks total
VW = HD + 1                # V tile width incl. ones column = 65

BF16 = mybir.dt.bfloat16
F32 = mybir.dt.float32
NPBF16 = ml_dtypes.bfloat16

_CACHE = {}


def _build():
    nc = bacc.Bacc("TRN2", target_bir_lowering=False, debug=False, num_devices=NCORES)
    xT = nc.dram_tensor("xT", [D, T], BF16, kind="ExternalInput").ap()
    wqT = nc.dram_tensor("wqT", [D, E], BF16, kind="ExternalInput").ap()
    wkT = nc.dram_tensor("wkT", [D, E], BF16, kind="ExternalInput").ap()
    wvT = nc.dram_tensor("wvT", [D, E], BF16, kind="ExternalInput").ap()
    wpT = nc.dram_tensor("wpT", [D, D], BF16, kind="ExternalInput").ap()
    mask = nc.dram_tensor("mask", [128, 1024], BF16, kind="ExternalInput").ap()
    out = nc.dram_tensor("out", [TQ, D], F32, kind="ExternalOutput").ap()

    with tile.TileContext(nc) as tc, ExitStack() as ctx:
        sing = ctx.enter_context(tc.tile_pool(name="sing", bufs=1))
        pwork = ctx.enter_context(tc.tile_pool(name="pwork", bufs=3))
        ynp = ctx.enter_context(tc.tile_pool(name="ynp", bufs=4))
        rp = ctx.enter_context(tc.tile_pool(name="rp", bufs=4))
        osb = ctx.enter_context(tc.tile_pool(name="osb", bufs=2))
        # PSUM budget (8 banks): psS 2 x [128,1024] (2 banks each) = 4;
        # psY 4 x [<=128,512] (1 bank each) = 4.
        psS = ctx.enter_context(tc.tile_pool(name="psS", bufs=2, space="PSUM"))
        psY = ctx.enter_context(tc.tile_pool(name="psY", bufs=4, space="PSUM"))
        dram = ctx.enter_context(tc.tile_pool(name="dram", bufs=1, space="DRAM"))

        # ---- resident SBUF tensors -------------------------------------
        xT_sb = sing.tile([128, KD * T], BF16)      # d-chunk kc at cols [kc*T, (kc+1)*T)
        wq_sb = sing.tile([128, KD * E], BF16)
        wk_sb = sing.tile([128, KD * E], BF16)
        wv_sb = sing.tile([128, KD * E], BF16)
        wp_sb = sing.tile([128, KD * D], BF16)
        mask_sb = sing.tile([128, 1024], BF16)
        qT_sb = sing.tile([128, T], BF16)           # rows 0:64 head0, 64:128 head1
        kT_sb = sing.tile([128, T], BF16)
        v0_sb = sing.tile([128, NV * VW], BF16)     # V head0, chunk ci at [ci*65, +65), col 64 = ones
        v1_sb = sing.tile([128, NV * VW], BF16)
        ya_sb = sing.tile([128, KD * TQ], BF16)     # gathered y^T for my tq rows
        ones_sb = sing.tile([1, 128], F32)
        zb_sb = sing.tile([128, 1], F32)            # zero bias for activations

        nc.vector.memset(ones_sb[:], 1.0)
        nc.vector.memset(zb_sb[:], 0.0)
        nc.vector.memset(
            v0_sb[:].rearrange("p (c w) -> p c w", w=VW)[:, :, HD : HD + 1], 1.0
        )
        nc.vector.memset(
            v1_sb[:].rearrange("p (c w) -> p c w", w=VW)[:, :, HD : HD + 1], 1.0
        )

        # ---- input DMAs -------------------------------------------------
        # xT loaded t-slice-major so t-block tb only depends on DMA tb.
        for tb in range(NB):
            nc.sync.dma_start(
                out=xT_sb[:].rearrange("p (c t) -> p c t", c=KD)[
                    :, :, tb * TQ : (tb + 1) * TQ
                ],
                in_=xT[:, tb * TQ : (tb + 1) * TQ].rearrange("(c p) t -> p c t", p=128),
            )
        for w_sb, w_dram, width in (
            (wq_sb, wqT, E),
            (wk_sb, wkT, E),
            (wv_sb, wvT, E),
            (wp_sb, wpT, D),
        ):
            nc.sync.dma_start(
                out=w_sb[:].rearrange("p (c e) -> p c e", c=KD),
                in_=w_dram.rearrange("(c p) e -> p c e", p=128),
            )
        nc.sync.dma_start(out=mask_sb[:], in_=mask)

        # ---- phase 1: QKV ----------------------------------------------
        # q^T, k^T: [e(2 heads)=128, T] accumulated over 8 d-chunks.
        for tb in range(NB):
            ts = tb * TQ
            psq = psS.tile([128, 1024], F32, tag="ps")
            psk = psS.tile([128, 1024], F32, tag="ps")
            for kc in range(KD):
                nc.tensor.matmul(
                    out=psq[:, 0:TQ],
                    lhsT=wq_sb[:, kc * E : (kc + 1) * E],
                    rhs=xT_sb[:, kc * T + ts : kc * T + ts + TQ],
                    start=(kc == 0),
                    stop=(kc == KD - 1),
                )
            for kc in range(KD):
                nc.tensor.matmul(
                    out=psk[:, 0:TQ],
                    lhsT=wk_sb[:, kc * E : (kc + 1) * E],
                    rhs=xT_sb[:, kc * T + ts : kc * T + ts + TQ],
                    start=(kc == 0),
                    stop=(kc == KD - 1),
                )
            nc.scalar.copy(out=qT_sb[:, ts : ts + TQ], in_=psq[:, 0:TQ])
            nc.vector.tensor_copy(out=kT_sb[:, ts : ts + TQ], in_=psk[:, 0:TQ])
        # V in natural [t, e] layout, both heads at once.
        for ci in range(NV):
            psv = psY.tile([128, 512], F32, tag="py")
            for kc in range(KD):
                nc.tensor.matmul(
                    out=psv[:, 0:E],
                    lhsT=xT_sb[:, kc * T + ci * CK : kc * T + (ci + 1) * CK],
                    rhs=wv_sb[:, kc * E : (kc + 1) * E],
                    start=(kc == 0),
                    stop=(kc == KD - 1),
                )
            eng = nc.scalar if ci % 2 == 0 else None
            if eng is not None:
                eng.copy(out=v0_sb[:, ci * VW : ci * VW + HD], in_=psv[:, 0:HD])
            else:
                nc.vector.tensor_copy(
                    out=v0_sb[:, ci * VW : ci * VW + HD], in_=psv[:, 0:HD]
                )
            nc.vector.tensor_copy(
                out=v1_sb[:, ci * VW : ci * VW + HD], in_=psv[:, HD:E]
            )

        # ---- phase 2: attention (per tq block) --------------------------
        send_t = dram.tile([NCORES, 128, TQ], BF16)
        recv_t = dram.tile([NCORES, 128, TQ], BF16)

        for b in range(NB):
            ts = b * TQ
            nchunks = 4 * (b + 1)          # tk chunks 0 .. (b+1)*512/128
            y0 = psY.tile([VW, TQ], F32, tag="py")
            y1 = psY.tile([VW, TQ], F32, tag="py")
            for sc in range(nchunks // 2):  # super-chunks of 2 tk chunks
                s0 = psS.tile([128, 1024], F32, tag="ps")
                s1 = psS.tile([128, 1024], F32, tag="ps")
                for j in range(2):
                    ci = 2 * sc + j
                    nc.tensor.matmul(
                        out=s0[:, j * TQ : (j + 1) * TQ],
                        lhsT=kT_sb[0:HD, ci * CK : (ci + 1) * CK],
                        rhs=qT_sb[0:HD, ts : ts + TQ],
                        start=True,
                        stop=True,
                    )
                    nc.tensor.matmul(
                        out=s1[:, j * TQ : (j + 1) * TQ],
                        lhsT=kT_sb[HD:128, ci * CK : (ci + 1) * CK],
                        rhs=qT_sb[HD:128, ts : ts + TQ],
                        start=True,
                        stop=True,
                    )
                p0 = pwork.tile([128, 1024], BF16, tag="pt")
                p1 = pwork.tile([128, 1024], BF16, tag="pt")
                nc.scalar.activation(
                    out=p0[:], in_=s0[:], func=mybir.ActivationFunctionType.Exp,
                    bias=zb_sb[:],
                )
                nc.scalar.activation(
                    out=p1[:], in_=s1[:], func=mybir.ActivationFunctionType.Exp,
                    bias=zb_sb[:],
                )
                for j in range(2):
                    ci = 2 * sc + j
                    off = ci * CK - ts
                    if off >= 0:           # diagonal chunk: causal mask
                        ms = mask_sb[:, 512 - off : 1024 - off]
                        nc.vector.tensor_mul(
                            p0[:, j * TQ : (j + 1) * TQ],
                            p0[:, j * TQ : (j + 1) * TQ],
                            ms,
                        )
                        nc.vector.tensor_mul(
                            p1[:, j * TQ : (j + 1) * TQ],
                            p1[:, j * TQ : (j + 1) * TQ],
                            ms,
                        )
                for j in range(2):
                    ci = 2 * sc + j
                    nc.tensor.matmul(
                        out=y0[:],
                        lhsT=v0_sb[:, ci * VW : (ci + 1) * VW],
                        rhs=p0[:, j * TQ : (j + 1) * TQ],
                        start=(ci == 0),
                        stop=(ci == nchunks - 1),
                    )
                    nc.tensor.matmul(
                        out=y1[:],
                        lhsT=v1_sb[:, ci * VW : (ci + 1) * VW],
                        rhs=p1[:, j * TQ : (j + 1) * TQ],
                        start=(ci == 0),
                        stop=(ci == nchunks - 1),
                    )
            # normalize: y[0:64] / y[64] per head, write to the all-to-all
            # send slot for destination core b.
            r0 = rp.tile([1, TQ], F32, tag="r")
            r1 = rp.tile([1, TQ], F32, tag="r")
            nc.vector.reciprocal(out=r0[:], in_=y0[HD : HD + 1, :])
            nc.vector.reciprocal(out=r1[:], in_=y1[HD : HD + 1, :])
            rb0 = psY.tile([HD, TQ], F32, tag="py")
            rb1 = psY.tile([HD, TQ], F32, tag="py")
            nc.tensor.matmul(
                out=rb0[:], lhsT=ones_sb[:, 0:HD], rhs=r0[:], start=True, stop=True
            )
            nc.tensor.matmul(
                out=rb1[:], lhsT=ones_sb[:, 0:HD], rhs=r1[:], start=True, stop=True
            )
            rb0_sb = rp.tile([HD, TQ], F32, tag="rb")
            rb1_sb = rp.tile([HD, TQ], F32, tag="rb")
            nc.scalar.copy(out=rb0_sb[:], in_=rb0[:])
            nc.scalar.copy(out=rb1_sb[:], in_=rb1[:])
            yn0 = ynp.tile([HD, TQ], BF16, tag="yn")
            yn1 = ynp.tile([HD, TQ], BF16, tag="yn")
            nc.vector.tensor_mul(yn0[:], y0[0:HD, :], rb0_sb[:])
            nc.vector.tensor_mul(yn1[:], y1[0:HD, :], rb1_sb[:])
            nc.sync.dma_start(out=send_t[b, 0:HD, :], in_=yn0[:])
            nc.sync.dma_start(out=send_t[b, HD:128, :], in_=yn1[:])

        # ---- phase 3: all-to-all + output projection --------------------
        nc.gpsimd.collective_compute(
            "AllToAll",
            mybir.AluOpType.bypass,
            replica_groups=[list(range(NCORES))],
            ins=[send_t[:].opt()],
            outs=[recv_t[:].opt()],
        )
        for j in range(NCORES):
            nc.sync.dma_start(
                out=ya_sb[:, j * TQ : (j + 1) * TQ], in_=recv_t[j, :, :]
            )
        for mt in range(TQ // 128):
            out_sb = osb.tile([128, D], F32, tag="o")
            for nh in range(2):
                po = psS.tile([128, 1024], F32, tag="ps")
                for kc in range(KD):
                    nc.tensor.matmul(
                        out=po[:, 0:512],
                        lhsT=ya_sb[:, kc * TQ + mt * 128 : kc * TQ + (mt + 1) * 128],
                        rhs=wp_sb[:, kc * D + nh * 512 : kc * D + (nh + 1) * 512],
                        start=(kc == 0),
                        stop=(kc == KD - 1),
                    )
                if nh == 0:
                    nc.scalar.copy(out=out_sb[:, 0:512], in_=po[:, 0:512])
                else:
                    nc.vector.tensor_copy(out=out_sb[:, 512:1024], in_=po[:, 0:512])
            nc.sync.dma_start(out=out[mt * 128 : (mt + 1) * 128, :], in_=out_sb[:])

    nc.compile()
    return nc


def _inputs(x, w_attn, w_proj):
    x = np.asarray(x, dtype=np.float32).reshape(T, D)
    w_attn = np.asarray(w_attn, dtype=np.float32)
    w_proj = np.asarray(w_proj, dtype=np.float32)

    xT_np = np.ascontiguousarray(x.T).astype(NPBF16)
    wpT_np = np.ascontiguousarray(w_proj.T).astype(NPBF16)
    scale = 1.0 / math.sqrt(HD)
    p = np.arange(128)[:, None]
    c = np.arange(1024)[None, :]
    mask_np = (c >= p + 512).astype(NPBF16)

    in_maps = []
    for core in range(NCORES):
        r0 = core * E
        in_maps.append(
            {
                "xT": xT_np,
                "wqT": np.ascontiguousarray((w_attn[r0 : r0 + E, :] * scale).T).astype(
                    NPBF16
                ),
                "wkT": np.ascontiguousarray(w_attn[D + r0 : D + r0 + E, :].T).astype(
                    NPBF16
                ),
                "wvT": np.ascontiguousarray(
                    w_attn[2 * D + r0 : 2 * D + r0 + E, :].T
                ).astype(NPBF16),
                "wpT": wpT_np,
                "mask": mask_np,
            }
        )
    return in_maps


def kernel(x, w_attn, w_proj, _trace=False):
    if "nc" not in _CACHE:
        _CACHE["nc"] = _build()
    nc = _CACHE["nc"]
    in_maps = _inputs(x, w_attn, w_proj)
    res = run_bass_kernel_spmd(
        nc, in_maps, core_ids=list(range(NCORES)), trace=_trace
    )
    _CACHE["last_result"] = res
    full = np.concatenate([res.results[c]["out"] for c in range(NCORES)], axis=0)
    return full.reshape(B, T, D).astype(np.float32)


# revision 7
# speedup vs baseline: 1.1051x; 1.1051x over previous
"""Causal self-attention (B=1, T=4096, D=1024, H=16, HD=64) on 8 trn2 NeuronCores.

Sharding: tensor-parallel over heads (2 heads per core) for QKV + attention;
on-chip AllToAll re-shards to sequence-parallel for the output projection
(each core computes a 512-row slice of the output).

Matmul layout notes (PE computes out = lhsT.T @ rhs, contraction on partitions):
 - host feeds x transposed (xT [D, T]) so QKV needs no on-chip transposes.
 - S^T tiles [tk, tq] are computed (not S) so the PV matmul can consume
   exp(S^T) directly as the moving operand with V in natural [tk, hd] layout.
   The two heads' QK matmuls run concurrently via PE row tiling (K=64 each).
 - a ones-column appended to V makes row 64 of the PV accumulator the
   softmax denominator (no extra reduction pass).
 - softmax max-subtraction is skipped: scores are ~N(0,1) (|s| < ~10), and
   a constant shift cancels exactly in softmax, so exp is safe in fp32.
"""

import math
import sys
from contextlib import ExitStack

sys.path.insert(0, "/opt/trn_rl_repo")

import ml_dtypes
import numpy as np

import concourse.bass as bass  # noqa: F401  (bass types used via tile/bacc)
import concourse.mybir as mybir
import concourse.tile as tile
from concourse import bacc
from concourse.bass_utils import run_bass_kernel_spmd

B, T, D, H, HD = 1, 4096, 1024, 16, 64
NCORES = 8
HPC = H // NCORES          # heads per core = 2
E = HPC * HD               # per-core head width = 128
TQ = 512                   # tq block width
NB = T // TQ               # 8 tq blocks
CK = 128                   # tk chunk (partition dim of S^T tiles)
KD = D // 128              # 8 contraction chunks over D
NV = T // CK               # 32 tk chunks total
VW = HD + 1                # V tile width incl. ones column = 65

BF16 = mybir.dt.bfloat16
F32 = mybir.dt.float32
NPBF16 = ml_dtypes.bfloat16

_CACHE = {}


def _build():
    nc = bacc.Bacc("TRN2", target_bir_lowering=False, debug=False, num_devices=NCORES)
    xT = nc.dram_tensor("xT", [D, T], BF16, kind="ExternalInput").ap()
    wqT = nc.dram_tensor("wqT", [D, E], BF16, kind="ExternalInput").ap()
    wkT = nc.dram_tensor("wkT", [D, E], BF16, kind="ExternalInput").ap()
    wvT = nc.dram_tensor("wvT", [D, E], BF16, kind="ExternalInput").ap()
    wpT = nc.dram_tensor("wpT", [D, D], BF16, kind="ExternalInput").ap()
    mask = nc.dram_tensor("mask", [128, 1024], BF16, kind="ExternalInput").ap()
    out = nc.dram_tensor("out", [TQ, D], F32, kind="ExternalOutput").ap()

    with tile.TileContext(nc) as tc, ExitStack() as ctx:
        sing = ctx.enter_context(tc.tile_pool(name="sing", bufs=1))
        pwork = ctx.enter_context(tc.tile_pool(name="pwork", bufs=3))
        ynp = ctx.enter_context(tc.tile_pool(name="ynp", bufs=4))
        osb = ctx.enter_context(tc.tile_pool(name="osb", bufs=2))
        # PSUM budget (8 banks): psS 2 x [128,1024] (2 banks each) = 4;
        # psY 4 x [<=128,512] (1 bank each) = 4 (y0/y1 + pipelining/bcast).
        psS = ctx.enter_context(tc.tile_pool(name="psS", bufs=2, space="PSUM"))
        psY = ctx.enter_context(tc.tile_pool(name="psY", bufs=4, space="PSUM"))
        dram = ctx.enter_context(tc.tile_pool(name="dram", bufs=1, space="DRAM"))

        # ---- resident SBUF tensors -------------------------------------
        xT_sb = sing.tile([128, KD * T], BF16)      # d-chunk kc at cols [kc*T, (kc+1)*T)
        wq_sb = sing.tile([128, KD * E], BF16)
        wk_sb = sing.tile([128, KD * E], BF16)
        wv_sb = sing.tile([128, KD * E], BF16)
        wp_sb = sing.tile([128, KD * D], BF16)
        mask_sb = sing.tile([128, 1024], BF16)
        qT_sb = sing.tile([128, T], BF16)           # rows 0:64 head0, 64:128 head1
        kT_sb = sing.tile([128, T], BF16)
        v0_sb = sing.tile([128, NV * VW], BF16)     # V head0, chunk ci at [ci*65, +65), col 64 = ones
        v1_sb = sing.tile([128, NV * VW], BF16)
        ya_sb = sing.tile([128, KD * TQ], BF16)     # gathered y^T for my tq rows
        y_sb = sing.tile([128, T], F32)             # unnormalized y^T (both heads)
        dsp_sb = sing.tile([128, 4 * TQ], F32)      # den rows at partitions {0,32,64,96}
        onesp_sb = sing.tile([128, 128], F32)
        zb_sb = sing.tile([128, 1], F32)            # zero bias for activations

        nc.vector.memset(zb_sb[:], 0.0)
        nc.vector.memset(onesp_sb[:], 1.0)
        nc.vector.memset(
            v0_sb[:].rearrange("p (c w) -> p c w", w=VW)[:, :, HD : HD + 1], 1.0
        )
        nc.vector.memset(
            v1_sb[:].rearrange("p (c w) -> p c w", w=VW)[:, :, HD : HD + 1], 1.0
        )

        # ---- input DMAs -------------------------------------------------
        # small weights first (they gate the first matmuls), then xT
        # t-slice-major so QKV t-block tb only depends on xT DMA tb.
        for w_sb, w_dram in ((wq_sb, wqT), (wk_sb, wkT), (wv_sb, wvT)):
            nc.sync.dma_start(
                out=w_sb[:].rearrange("p (c e) -> p c e", c=KD),
                in_=w_dram.rearrange("(c p) e -> p c e", p=128),
            )
        nc.scalar.dma_start(out=mask_sb[:], in_=mask)
        for tb in range(NB):
            nc.sync.dma_start(
                out=xT_sb[:].rearrange("p (c t) -> p c t", c=KD)[
                    :, :, tb * TQ : (tb + 1) * TQ
                ],
                in_=xT[:, tb * TQ : (tb + 1) * TQ].rearrange("(c p) t -> p c t", p=128),
            )
        nc.scalar.dma_start(
            out=wp_sb[:].rearrange("p (c e) -> p c e", c=KD),
            in_=wpT.rearrange("(c p) e -> p c e", p=128),
        )

        # ---- phase 1: QKV ----------------------------------------------
        # emitted per t-block (q, k, then that block's V chunks) so the
        # attention pipeline for block b can start as soon as its inputs
        # exist instead of after the whole QKV phase.
        for tb in range(NB):
            ts = tb * TQ
            psq = psS.tile([128, 1024], F32, tag="ps")
            psk = psS.tile([128, 1024], F32, tag="ps")
            for kc in range(KD):
                nc.tensor.matmul(
                    out=psq[:, 0:TQ],
                    lhsT=wq_sb[:, kc * E : (kc + 1) * E],
                    rhs=xT_sb[:, kc * T + ts : kc * T + ts + TQ],
                    start=(kc == 0),
                    stop=(kc == KD - 1),
                )
            for kc in range(KD):
                nc.tensor.matmul(
                    out=psk[:, 0:TQ],
                    lhsT=wk_sb[:, kc * E : (kc + 1) * E],
                    rhs=xT_sb[:, kc * T + ts : kc * T + ts + TQ],
                    start=(kc == 0),
                    stop=(kc == KD - 1),
                )
            nc.scalar.copy(out=qT_sb[:, ts : ts + TQ], in_=psq[:, 0:TQ])
            nc.vector.tensor_copy(out=kT_sb[:, ts : ts + TQ], in_=psk[:, 0:TQ])
            for ci in range(4 * tb, 4 * tb + 4):   # V chunks for this t-block
                psv = psY.tile([128, 512], F32, tag="py")
                for kc in range(KD):
                    nc.tensor.matmul(
                        out=psv[:, 0:E],
                        lhsT=xT_sb[:, kc * T + ci * CK : kc * T + (ci + 1) * CK],
                        rhs=wv_sb[:, kc * E : (kc + 1) * E],
                        start=(kc == 0),
                        stop=(kc == KD - 1),
                    )
                nc.vector.tensor_copy(
                    out=v0_sb[:, ci * VW : ci * VW + HD], in_=psv[:, 0:HD]
                )
                nc.scalar.copy(
                    out=v1_sb[:, ci * VW : ci * VW + HD], in_=psv[:, HD:E]
                )

        # ---- phase 2: attention (per tq block) --------------------------
        send_t = dram.tile([NCORES, 128, TQ], BF16)
        recv_t = dram.tile([NCORES, 128, TQ], BF16)

        for b in range(NB):
            ts = b * TQ
            nchunks = 4 * (b + 1)          # tk chunks 0 .. (b+1)*512/128
            y0 = psY.tile([VW, TQ], F32, tag="py")
            y1 = psY.tile([VW, TQ], F32, tag="py")
            for sc in range(nchunks // 2):  # super-chunks of 2 tk chunks
                s0 = psS.tile([128, 1024], F32, tag="ps")
                s1 = psS.tile([128, 1024], F32, tag="ps")
                for j in range(2):
                    ci = 2 * sc + j
                    nc.tensor.matmul(
                        out=s0[:, j * TQ : (j + 1) * TQ],
                        lhsT=kT_sb[0:HD, ci * CK : (ci + 1) * CK],
                        rhs=qT_sb[0:HD, ts : ts + TQ],
                        start=True,
                        stop=True,
                    )
                    nc.tensor.matmul(
                        out=s1[:, j * TQ : (j + 1) * TQ],
                        lhsT=kT_sb[HD:128, ci * CK : (ci + 1) * CK],
                        rhs=qT_sb[HD:128, ts : ts + TQ],
                        start=True,
                        stop=True,
                    )
                p0 = pwork.tile([128, 1024], BF16, tag="pt")
                p1 = pwork.tile([128, 1024], BF16, tag="pt")
                nc.scalar.activation(
                    out=p0[:], in_=s0[:], func=mybir.ActivationFunctionType.Exp,
                    bias=zb_sb[:],
                )
                nc.scalar.activation(
                    out=p1[:], in_=s1[:], func=mybir.ActivationFunctionType.Exp,
                    bias=zb_sb[:],
                )
                for j in range(2):
                    ci = 2 * sc + j
                    off = ci * CK - ts
                    if off >= 0:           # diagonal chunk: causal mask
                        ms = mask_sb[:, 512 - off : 1024 - off]
                        nc.vector.tensor_mul(
                            p0[:, j * TQ : (j + 1) * TQ],
                            p0[:, j * TQ : (j + 1) * TQ],
                            ms,
                        )
                        nc.vector.tensor_mul(
                            p1[:, j * TQ : (j + 1) * TQ],
                            p1[:, j * TQ : (j + 1) * TQ],
                            ms,
                        )
                for j in range(2):
                    ci = 2 * sc + j
                    nc.tensor.matmul(
                        out=y0[:],
                        lhsT=v0_sb[:, ci * VW : (ci + 1) * VW],
                        rhs=p0[:, j * TQ : (j + 1) * TQ],
                        start=(ci == 0),
                        stop=(ci == nchunks - 1),
                    )
                    nc.tensor.matmul(
                        out=y1[:],
                        lhsT=v1_sb[:, ci * VW : (ci + 1) * VW],
                        rhs=p1[:, j * TQ : (j + 1) * TQ],
                        start=(ci == 0),
                        stop=(ci == nchunks - 1),
                    )
            # stash unnormalized y + denominators; normalization is deferred
            # past the last block so it never stalls the attention pipeline.
            nc.vector.tensor_copy(out=y_sb[0:HD, ts : ts + TQ], in_=y0[0:HD, :])
            nc.scalar.copy(out=y_sb[HD:128, ts : ts + TQ], in_=y1[0:HD, :])
            for h, yy in ((0, y0), (1, y1)):
                i = 2 * b + h
                nc.vector.tensor_copy(
                    out=dsp_sb[
                        (i % 4) * 32 : (i % 4) * 32 + 1,
                        (i // 4) * TQ : (i // 4 + 1) * TQ,
                    ],
                    in_=yy[HD : HD + 1, :],
                )

        # ---- deferred softmax normalization -----------------------------
        # Per block: K=1 matmuls broadcast the denominator rows (stashed at
        # 32-aligned partitions - PE base constraint) across partitions, then
        # fast reciprocal + multiply. Runs after the attention pipeline so it
        # never stalls it.
        for b in range(NB):
            rb = psY.tile([128, TQ], F32, tag="py")
            for h in range(2):
                i = 2 * b + h
                pr = (i % 4) * 32
                cr = (i // 4) * TQ
                nc.tensor.matmul(
                    out=rb[h * HD : (h + 1) * HD, :],
                    lhsT=onesp_sb[pr : pr + 1, 0:HD],
                    rhs=dsp_sb[pr : pr + 1, cr : cr + TQ],
                    start=True,
                    stop=True,
                    tile_position=(pr, h * HD),
                )
            rq = ynp.tile([128, TQ], F32, tag="rq")
            nc.vector.reciprocal_approx_fast(out=rq[:], in_=rb[:])
            yn = ynp.tile([128, TQ], BF16, tag="yn")
            nc.vector.tensor_mul(yn[:], y_sb[:, b * TQ : (b + 1) * TQ], rq[:])
            nc.sync.dma_start(out=send_t[b, :, :], in_=yn[:])

        # ---- phase 3: all-to-all + output projection --------------------
        nc.gpsimd.collective_compute(
            "AllToAll",
            mybir.AluOpType.bypass,
            replica_groups=[list(range(NCORES))],
            ins=[send_t[:].opt()],
            outs=[recv_t[:].opt()],
        )
        for j in range(NCORES):
            nc.sync.dma_start(
                out=ya_sb[:, j * TQ : (j + 1) * TQ], in_=recv_t[j, :, :]
            )
        for mt in range(TQ // 128):
            out_sb = osb.tile([128, D], F32, tag="o")
            for nh in range(2):
                po = psS.tile([128, 1024], F32, tag="ps")
                for kc in range(KD):
                    nc.tensor.matmul(
                        out=po[:, 0:512],
                        lhsT=ya_sb[:, kc * TQ + mt * 128 : kc * TQ + (mt + 1) * 128],
                        rhs=wp_sb[:, kc * D + nh * 512 : kc * D + (nh + 1) * 512],
                        start=(kc == 0),
                        stop=(kc == KD - 1),
                    )
                if nh == 0:
                    nc.scalar.copy(out=out_sb[:, 0:512], in_=po[:, 0:512])
                else:
                    nc.vector.tensor_copy(out=out_sb[:, 512:1024], in_=po[:, 0:512])
            nc.sync.dma_start(out=out[mt * 128 : (mt + 1) * 128, :], in_=out_sb[:])

    nc.compile()
    return nc


def _inputs(x, w_attn, w_proj):
    x = np.asarray(x, dtype=np.float32).reshape(T, D)
    w_attn = np.asarray(w_attn, dtype=np.float32)
    w_proj = np.asarray(w_proj, dtype=np.float32)

    xT_np = np.ascontiguousarray(x.T).astype(NPBF16)
    wpT_np = np.ascontiguousarray(w_proj.T).astype(NPBF16)
    scale = 1.0 / math.sqrt(HD)
    p = np.arange(128)[:, None]
    c = np.arange(1024)[None, :]
    mask_np = (c >= p + 512).astype(NPBF16)

    in_maps = []
    for core in range(NCORES):
        r0 = core * E
        in_maps.append(
            {
                "xT": xT_np,
                "wqT": np.ascontiguousarray((w_attn[r0 : r0 + E, :] * scale).T).astype(
                    NPBF16
                ),
                "wkT": np.ascontiguousarray(w_attn[D + r0 : D + r0 + E, :].T).astype(
                    NPBF16
                ),
                "wvT": np.ascontiguousarray(
                    w_attn[2 * D + r0 : 2 * D + r0 + E, :].T
                ).astype(NPBF16),
                "wpT": wpT_np,
                "mask": mask_np,
            }
        )
    return in_maps


def kernel(x, w_attn, w_proj, _trace=False):
    if "nc" not in _CACHE:
        _CACHE["nc"] = _build()
    nc = _CACHE["nc"]
    in_maps = _inputs(x, w_attn, w_proj)
    res = run_bass_kernel_spmd(
        nc, in_maps, core_ids=list(range(NCORES)), trace=_trace
    )
    _CACHE["last_result"] = res
    full = np.concatenate([res.results[c]["out"] for c in range(NCORES)], axis=0)
    return full.reshape(B, T, D).astype(np.float32)
